# revision 48
# baseline (speedup 1.0000x reference)
"""CORLoss Trainium2 kernel (v2).

Reference (per row of N=128):
    mean1 = mean(d1) + EPS ; mean2 = mean(d2) + EPS
    std1, std2 unbiased ; cov = sum((d1-mean1)*(d2-mean2))/(n-1)
    cor  = (cov / (std1*std2 + EPS)) ** 3
    tl1  = -log((cor + 1 + EPS)/2)
    tl2  = mean(|softmax(d1) - softmax(d2)|)
    a = |cor| ; loss_row = a*tl1 + (1-a)*tl2
    out  = sum(loss_row) over all B rows, shape (1,)

Strategy: data-parallel over 8 NeuronCores, 16384 rows/core, streamed as
[128 partitions, 16 blocks, 128] supertiles (one row per (partition,
block)).  Engine assignment per supertile (2048 elems/lane):

  DVE : bn_stats(d1), bn_stats(d2)  -> per-row (mean, M2) via even/odd
        merge in the epilogue; fp16 2x fold tails for the se / s12 / M
        sums; min(e1, f).
  ACT : e1 = Exp(d1), e2 = Exp(d2) (fp16); f = c*e2 computed per
        128-col block as Exp(d2 + ln c) with a per-partition bias (rows
        == partitions inside one block); all funcs live in the
        natural_log_exp_and_others table so it loads once.
  Pool: p12 = d1*d2 (scalar_tensor_tensor: 1.39ns/elem vs 1.98 for
        tensor_tensor), and fold level-1 of the [e1,e2] and [p12,min]
        stream pairs.

  |p - q| uses Σ|a-b| = Σa + Σb - 2Σmin(a,b) (a,b >= 0):
        T = Σ|e1 - c*e2| = 2*se1 - 2*Σmin(e1, c*e2)
  so no Abs pass and the summed stream is non-negative fp16.

Per-row epilogue merges bn even/odd stats (Chan), forms cor, tl1, tl2,
the loss, and one [128,1] partial sum per core; the host adds 8*128
partials.  sqrt(w) is exp(0.5*ln w) to stay in the one ACT table.
"""

import sys

sys.path.insert(0, "/opt/trn_rl_repo")

import numpy as np

import concourse.bass as bass
import concourse.tile as tile
from concourse import mybir

B, N = 131072, 128
EPS = 1e-3
N_CORES = 8
R = B // N_CORES          # rows per core = 16384
ST_ROWS = 2048            # rows per supertile
NB = ST_ROWS // 128       # 16 row-blocks per supertile
NST = R // ST_ROWS        # 8 supertiles per core
NCOLS = R // 128          # 128 stat columns per core
F32 = mybir.dt.float32
FP16 = mybir.dt.float16
Alu = mybir.AluOpType
Act = mybir.ActivationFunctionType


ABLATE = set()  # timing probes only; populated externally


def _tt(nc, out, a, b, op):
    nc.vector.tensor_tensor(out=out, in0=a, in1=b, op=op)


def split_waits(nc, cap=1):
    """This walrus build rejects instructions carrying more than ~1 inline
    semaphore wait; move excess waits onto fresh same-engine nops placed
    immediately before the instruction."""
    for fn in nc.m.functions:
        for bb in fn.blocks:
            snapshot = list(bb.instructions)
            out = []
            for inst in snapshot:
                si = inst.sync_info
                if si is not None and si.on_wait and len(si.on_wait) > cap:
                    waits = list(si.on_wait)
                    extra, keep = waits[:-cap], waits[-cap:]
                    while si.on_wait:
                        si.on_wait.pop()
                    for w in keep:
                        si.on_wait.append(w)
                    for w in extra:
                        bi = nc.engines[inst.engine].nop(nofuse=True, hint="wsplit")
                        nop_inst = bi.ins
                        for fb in nc.m.functions[0].blocks:
                            if fb.instructions and fb.instructions[-1] is nop_inst:
                                fb.instructions.pop()
                                break
                        nop_inst.sync_info = mybir.SyncInfo(on_wait=[w], on_update=[])
                        out.append(nop_inst)
                out.append(inst)
            bb.instructions[:] = out


def build_body(
    nc, tc, d1, d2, y,
    data_pool, data2_pool, e_pool, f_pool, pm_pool, hse_pool, hpm_pool,
    small_pool, stats_pool, epi_pool,
):
    # persistent per-core stat arrays, one column per row-block.
    # bnst*: (count, mean, M2) per block-column, written two rows per
    # bn_stats instruction (see bn_pair below).
    bnst1 = stats_pool.tile([128, NCOLS, 3], F32, tag="bnst1", name="bnst1")
    bnst2 = stats_pool.tile([128, NCOLS, 3], F32, tag="bnst2", name="bnst2")
    sePair = stats_pool.tile([128, 2, NCOLS], F32, tag="sePair", name="sePair")
    pmPair = stats_pool.tile([128, 2, NCOLS], F32, tag="pmPair", name="pmPair")

    def pool_tt(out, in0, in1, op):
        nc.gpsimd.tensor_tensor(out=out, in0=in0, in1=in1, op=op)

    def bn_pair(t, bnst, col0):
        """One bn_stats over rows (col0, col0+1) of the supertile: the
        input AP interleaves the two 128-col rows element-by-element
        (transpose puts the block dim fastest), so the instruction's
        even elements are row A and its odd elements are row B.  The HW
        computes (count, mean, count*var) for each parity — per-row
        stats, two rows per instruction.  The [128, 2, 3] output AP
        lands them as (count, mean, M2) per block-column.  Emitted raw:
        the bass wrapper's segment-shape assert predates this AP use."""
        in_ap = t[:, col0 % NB : col0 % NB + 2, :].transpose([0, 2, 1])
        out_ap = bnst[:, col0 : col0 + 2, :]
        nc.vector.add_instruction(
            mybir.InstBNStats(
                name=nc.get_next_instruction_name(),
                ins=[nc.vector.lower_ap(in_ap)],
                outs=[nc.vector.lower_ap(out_ap)],
            )
        )

    def fold_tail(src_h1, out_cols, chain):
        """src_h1 [128,2,NB,64] fp16 -> out_cols [128,2,NB] f32 via two
        fp16 2x halving adds + one segmented reduce."""
        hp = hse_pool if chain == "se" else hpm_pool
        h2 = hp.tile([128, 2, NB, 32], FP16, tag=f"h2{chain}", name=f"h2{chain}")
        _tt(nc, h2, src_h1[:, :, :, 0:32], src_h1[:, :, :, 32:64], Alu.add)
        h3 = hp.tile([128, 2, NB, 16], FP16, tag=f"h3{chain}", name=f"h3{chain}")
        _tt(nc, h3, h2[:, :, :, 0:16], h2[:, :, :, 16:32], Alu.add)
        nc.vector.reduce_sum(out=out_cols, in_=h3, axis=mybir.AxisListType.X)

    def stage_load(st):
        rows = slice(st * ST_ROWS, (st + 1) * ST_ROWS)
        src1 = d1[rows, :].rearrange("(p b) n -> p b n", p=128)
        src2 = d2[rows, :].rearrange("(p b) n -> p b n", p=128)
        t1 = data_pool.tile([128, NB, N], F32, tag="t1", name="t1")
        t2 = data2_pool.tile([128, NB, N], F32, tag="t2", name="t2")
        nc.sync.dma_start(out=t1, in_=src1)
        nc.sync.dma_start(out=t2, in_=src2)
        live[("t", st)] = (t1, t2)

    def stage_main(st):
        t1, t2 = live[("t", st)]
        e12 = e_pool.tile([128, 2, NB, N], FP16, tag="e12", name="e12")
        nc.scalar.activation(out=e12[:, 0], in_=t1, func=Act.Exp)
        nc.scalar.activation(out=e12[:, 1], in_=t2, func=Act.Exp)
        pm = pm_pool.tile([128, 2, NB, N], FP16, tag="pm", name="pm")
        _tt(nc, pm[:, 0], t1, t2, Alu.mult)
        for k in range(NB // 2):
            bn_pair(t1, bnst1, st * NB + 2 * k)
            bn_pair(t2, bnst2, st * NB + 2 * k)
        live[("e", st)] = (e12, pm)

    def stage_se(st):
        """se1/se2 via one direct fp16 segmented reduce, then c."""
        e12, pm = live[("e", st)]
        cols = slice(st * NB, (st + 1) * NB)
        nc.vector.reduce_sum(
            out=sePair[:, :, cols], in_=e12, axis=mybir.AxisListType.X
        )
        rse2 = small_pool.tile([128, NB], F32, tag="rse2", name="rse2")
        nc.vector.reciprocal(out=rse2, in_=sePair[:, 1, cols])
        cC = small_pool.tile([128, NB], F32, tag="cC", name="cC")
        _tt(nc, cC, sePair[:, 0, cols], rse2, Alu.mult)
        live[("c", st)] = cC

    def stage_f(st):
        """ln(c), then f = c*e2 = Exp(d2 + ln c) per block; f overwrites
        e12 channel 1 (e2's last reader was the se reduce)."""
        cC = live.pop(("c", st))
        _t1, t2 = live[("t", st)]
        e12, pm = live[("e", st)]
        lnc = small_pool.tile([128, NB], F32, tag="lnc", name="lnc")
        nc.scalar.activation(out=lnc, in_=cC, func=Act.Ln)
        f = f_pool.tile([128, NB, N], FP16, tag="f", name="f")
        for b in range(NB):
            nc.scalar.activation(
                out=f[:, b], in_=t2[:, b], func=Act.Exp,
                bias=lnc[:, b : b + 1],
            )
        live[("f", st)] = f

    def stage_pm_chain(st):
        """min then one direct fp16 reduce of the [p12, min] pack."""
        e12, pm = live.pop(("e", st))
        f = live.pop(("f", st))
        live.pop(("t", st))
        cols = slice(st * NB, (st + 1) * NB)
        _tt(nc, pm[:, 1], e12[:, 0], f, Alu.min)
        nc.vector.reduce_sum(
            out=pmPair[:, :, cols], in_=pm, axis=mybir.AxisListType.X
        )

    live = {}
    pre = [None]

    def epi_post():
        _emit_epilogue_post(nc, epi_pool, pre[0], pmPair, y)

    def valid(st):
        return 0 <= st <= NST - 1

    # lags: load k (LEAD cycles ahead of compute) | exp/p12/bn k-LEAD |
    # se k-LEAD-1 | f+min+pm k-LEAD-2
    LEAD = 1
    for k in range(NST + LEAD + 2):
        if valid(k - LEAD - 2) and "f" not in ABLATE:
            stage_f(k - LEAD - 2)
        if k < NST:
            stage_load(k)
        if valid(k - LEAD):
            stage_main(k - LEAD)
        if valid(k - LEAD - 1) and "se" not in ABLATE:
            stage_se(k - LEAD - 1)
        if k == NST + LEAD and not ABLATE:
            pre[0] = _emit_epilogue_pre(nc, epi_pool, bnst1, bnst2, sePair)
        if valid(k - LEAD - 2) and "pm" not in ABLATE and "f" not in ABLATE:
            stage_pm_chain(k - LEAD - 2)

    # ---- per-row epilogue on [128, NCOLS] stat arrays ----
    # (epi_pre was emitted into the pipeline drain above)
    if ABLATE:
        part = epi_pool.tile([128, 1], F32, tag="part", name="part")
        nc.vector.memset(part, 0.0)
        nc.sync.dma_start(out=y[:, :], in_=part)
    else:
        epi_post()


def _emit_epilogue_pre(nc, epi_pool, bnst1, bnst2, sePair):
    """Loss-chain ops that depend only on bn stats + se sums: emitted
    into the pipeline drain so they overlap the last pm chains."""
    def ep(name):
        return epi_pool.tile([128, NCOLS], F32, tag=name, name=name)

    Alu = mybir.AluOpType
    Act = mybir.ActivationFunctionType
    m1, m2_1 = bnst1[:, :, 1], bnst1[:, :, 2]
    m2, m2_2 = bnst2[:, :, 1], bnst2[:, :, 2]
    mm = ep("mm")
    _tt(nc, mm, m1, m2, Alu.mult)
    # den = sqrt(M2_1*M2_2) + (N-1)*eps ; sqrt via exp(0.5*ln(w)) so every
    # ACT func stays in the natural_log_exp_and_others table
    w, sqw = ep("w"), ep("sqw")
    _tt(nc, w, m2_1, m2_2, Alu.mult)
    nc.scalar.activation(out=sqw, in_=w, func=Act.Ln)
    nc.scalar.activation(out=sqw, in_=sqw, func=Act.Exp, scale=0.5)
    den, rden = ep("den"), ep("rden")
    nc.vector.tensor_scalar(
        out=den, in0=sqw, scalar1=(N - 1) * EPS, scalar2=None, op0=Alu.add
    )
    nc.vector.reciprocal(out=rden, in_=den)
    rse1 = ep("rse1")
    nc.vector.reciprocal(out=rse1, in_=sePair[:, 0, :])
    ln_bias = epi_pool.tile([128, 1], F32, tag="ln_bias", name="ln_bias")
    nc.vector.memset(ln_bias, 1.0 + EPS)
    return mm, rden, rse1, ln_bias


def _emit_epilogue_post(nc, epi_pool, pre, pmPair, y):
    def ep(name):
        return epi_pool.tile([128, NCOLS], F32, tag=name, name=name)

    Alu = mybir.AluOpType
    Act = mybir.ActivationFunctionType
    mm, rden, rse1, ln_bias = pre
    s12A = pmPair[:, 0, :]
    MA = pmPair[:, 1, :]

    # num = s12 - N*m1*m2 ; cor = (num + N*eps^2) * rden
    num, cor = ep("num"), ep("cor")
    nc.vector.scalar_tensor_tensor(
        out=num, in0=mm, scalar=-float(N), in1=s12A, op0=Alu.mult, op1=Alu.add
    )
    nc.vector.scalar_tensor_tensor(
        out=cor, in0=num, scalar=float(N) * EPS * EPS, in1=rden,
        op0=Alu.add, op1=Alu.mult,
    )
    c2, cor3 = ep("c2"), ep("cor3")
    _tt(nc, c2, cor, cor, Alu.mult)
    _tt(nc, cor3, c2, cor, Alu.mult)

    # a = |cor3| ; tl1 = ln2 - ln(cor3 + 1 + eps)
    aa, lg, tl1 = ep("aa"), ep("lg"), ep("tl1")
    nc.scalar.activation(out=aa, in_=cor3, func=Act.Abs)
    nc.scalar.activation(out=lg, in_=cor3, func=Act.Ln, bias=ln_bias)
    nc.vector.tensor_scalar(
        out=tl1, in0=lg, scalar1=-1.0, scalar2=float(np.log(2.0)),
        op0=Alu.mult, op1=Alu.add,
    )

    # tl2 = (2/N)*(1 - M/se1)
    vv, tl2 = ep("vv"), ep("tl2")
    _tt(nc, vv, MA, rse1, Alu.mult)
    nc.vector.tensor_scalar(
        out=tl2, in0=vv, scalar1=-2.0 / N, scalar2=2.0 / N,
        op0=Alu.mult, op1=Alu.add,
    )

    # loss = tl2 + a*(tl1 - tl2)
    dd, pp, loss = ep("dd"), ep("pp"), ep("loss")
    _tt(nc, dd, tl1, tl2, Alu.subtract)
    _tt(nc, pp, aa, dd, Alu.mult)
    _tt(nc, loss, tl2, pp, Alu.add)

    part = epi_pool.tile([128, 1], F32, tag="part", name="part")
    nc.vector.reduce_sum(out=part, in_=loss, axis=mybir.AxisListType.X)
    nc.sync.dma_start(out=y[:, :], in_=part)


def _enter_pools(stack, tc):
    names_bufs = [
        ("data", 4), ("data2", 6), ("e", 4), ("f", 2), ("pm", 4),
        ("hse", 1), ("hpm", 1), ("small", 4), ("stats", 1), ("epi", 1),
    ]
    return [
        stack.enter_context(tc.tile_pool(name=nm, bufs=bf))
        for nm, bf in names_bufs
    ]


def _build_program():
    from contextlib import ExitStack

    nc = bass.Bass()
    d1 = nc.dram_tensor("d1", [R, N], F32, kind="ExternalInput")
    d2 = nc.dram_tensor("d2", [R, N], F32, kind="ExternalInput")
    y = nc.dram_tensor("y", [128, 1], F32, kind="ExternalOutput")

    with tile.TileContext(nc) as tc:
        with ExitStack() as stack:
            pools = _enter_pools(stack, tc)
            build_body(nc, tc, d1, d2, y, *pools)

    split_waits(nc)
    return nc


_NC = None
_RUNNER = None


def _get_nc():
    global _NC
    if _NC is None:
        _NC = _build_program()
    return _NC


def _get_runner():
    """Compile the 8-core pjrt executable once and reuse across calls."""
    global _RUNNER
    if _RUNNER is not None:
        return _RUNNER
    import jax
    from jax.sharding import Mesh, PartitionSpec
    from jax.experimental.shard_map import shard_map
    from concourse.bass2jax import (
        _bass_exec_p,
        install_neuronx_cc_hook,
        partition_id_tensor,
    )

    install_neuronx_cc_hook()
    nc = _get_nc()
    partition_name = nc.partition_id_tensor.name if nc.partition_id_tensor else None
    in_names, out_names, out_avals, zero_outs = [], [], [], []
    for alloc in nc.m.functions[0].allocations:
        if not isinstance(alloc, mybir.MemoryLocationSet):
            continue
        name = alloc.memorylocations[0].name
        if alloc.kind == "ExternalInput":
            if name != partition_name:
                in_names.append(name)
        elif alloc.kind == "ExternalOutput":
            out_names.append(name)
            shape = tuple(alloc.tensor_shape)
            dtype = mybir.dt.np(alloc.dtype)
            out_avals.append(jax.core.ShapedArray(shape, dtype))
            zero_outs.append(np.zeros(shape, dtype))
    n_params = len(in_names)
    all_in_names = list(in_names) + out_names
    if partition_name is not None:
        all_in_names.append(partition_name)

    def _body(*args):
        operands = list(args)
        if partition_name is not None:
            operands.append(partition_id_tensor())
        outs = _bass_exec_p.bind(
            *operands,
            out_avals=tuple(out_avals),
            in_names=tuple(all_in_names),
            out_names=tuple(out_names),
            lowering_input_output_aliases=(),
            sim_require_finite=True,
            sim_require_nnan=True,
            nc=nc,
        )
        return tuple(outs)

    devices = jax.devices()[:N_CORES]
    mesh = Mesh(np.asarray(devices), ("core",))
    n_outs = len(out_names)
    in_specs = (PartitionSpec("core"),) * (n_params + n_outs)
    out_specs = (PartitionSpec("core"),) * n_outs
    sharded = jax.jit(
        shard_map(
            _body, mesh=mesh, in_specs=in_specs, out_specs=out_specs,
            check_rep=False,
        ),
        keep_unused=True,
    )
    zero_cat = [
        np.zeros((N_CORES * z.shape[0], *z.shape[1:]), z.dtype) for z in zero_outs
    ]

    def run(d1, d2):
        ins = {"d1": d1, "d2": d2}
        out = sharded(*(ins[nm] for nm in in_names), *zero_cat)
        y = np.asarray(out[out_names.index("y")])
        return y

    _RUNNER = run
    return _RUNNER


def kernel(distribution1, distribution2):
    d1 = np.ascontiguousarray(np.asarray(distribution1, dtype=np.float32))
    d2 = np.ascontiguousarray(np.asarray(distribution2, dtype=np.float32))
    assert d1.shape == (B, N) and d2.shape == (B, N)
    y = _get_runner()(d1, d2)  # [N_CORES*128, 1] partial sums
    return np.asarray([np.sum(y.astype(np.float64))], dtype=np.float32)


# revision 49
# speedup vs baseline: 1.1049x; 1.1049x over previous
"""CORLoss Trainium2 kernel (v2).

Reference (per row of N=128):
    mean1 = mean(d1) + EPS ; mean2 = mean(d2) + EPS
    std1, std2 unbiased ; cov = sum((d1-mean1)*(d2-mean2))/(n-1)
    cor  = (cov / (std1*std2 + EPS)) ** 3
    tl1  = -log((cor + 1 + EPS)/2)
    tl2  = mean(|softmax(d1) - softmax(d2)|)
    a = |cor| ; loss_row = a*tl1 + (1-a)*tl2
    out  = sum(loss_row) over all B rows, shape (1,)

Strategy: data-parallel over 8 NeuronCores, 16384 rows/core, streamed as
[128 partitions, 16 blocks, 128] supertiles (one row per (partition,
block)).  Engine assignment per supertile (2048 elems/lane):

  DVE : bn_stats(d1), bn_stats(d2)  -> per-row (mean, M2) via even/odd
        merge in the epilogue; fp16 2x fold tails for the se / s12 / M
        sums; min(e1, f).
  ACT : e1 = Exp(d1), e2 = Exp(d2) (fp16); f = c*e2 computed per
        128-col block as Exp(d2 + ln c) with a per-partition bias (rows
        == partitions inside one block); all funcs live in the
        natural_log_exp_and_others table so it loads once.
  Pool: p12 = d1*d2 (scalar_tensor_tensor: 1.39ns/elem vs 1.98 for
        tensor_tensor), and fold level-1 of the [e1,e2] and [p12,min]
        stream pairs.

  |p - q| uses Σ|a-b| = Σa + Σb - 2Σmin(a,b) (a,b >= 0):
        T = Σ|e1 - c*e2| = 2*se1 - 2*Σmin(e1, c*e2)
  so no Abs pass and the summed stream is non-negative fp16.

Per-row epilogue merges bn even/odd stats (Chan), forms cor, tl1, tl2,
the loss, and one [128,1] partial sum per core; the host adds 8*128
partials.  sqrt(w) is exp(0.5*ln w) to stay in the one ACT table.
"""

import sys

sys.path.insert(0, "/opt/trn_rl_repo")

import numpy as np

import concourse.bass as bass
import concourse.tile as tile
from concourse import mybir

B, N = 131072, 128
EPS = 1e-3
N_CORES = 8
R = B // N_CORES          # rows per core = 16384
ST_ROWS = 2048            # rows per supertile
NB = ST_ROWS // 128       # 16 row-blocks per supertile
NST = R // ST_ROWS        # 8 supertiles per core
NCOLS = R // 128          # 128 stat columns per core
F32 = mybir.dt.float32
FP16 = mybir.dt.float16
Alu = mybir.AluOpType
Act = mybir.ActivationFunctionType


ABLATE = set()  # timing probes only; populated externally


def _tt(nc, out, a, b, op):
    nc.vector.tensor_tensor(out=out, in0=a, in1=b, op=op)


def split_waits(nc, cap=1):
    """This walrus build rejects instructions carrying more than ~1 inline
    semaphore wait; move excess waits onto fresh same-engine nops placed
    immediately before the instruction."""
    for fn in nc.m.functions:
        for bb in fn.blocks:
            snapshot = list(bb.instructions)
            out = []
            for inst in snapshot:
                si = inst.sync_info
                if si is not None and si.on_wait and len(si.on_wait) > cap:
                    waits = list(si.on_wait)
                    extra, keep = waits[:-cap], waits[-cap:]
                    while si.on_wait:
                        si.on_wait.pop()
                    for w in keep:
                        si.on_wait.append(w)
                    for w in extra:
                        bi = nc.engines[inst.engine].nop(nofuse=True, hint="wsplit")
                        nop_inst = bi.ins
                        for fb in nc.m.functions[0].blocks:
                            if fb.instructions and fb.instructions[-1] is nop_inst:
                                fb.instructions.pop()
                                break
                        nop_inst.sync_info = mybir.SyncInfo(on_wait=[w], on_update=[])
                        out.append(nop_inst)
                out.append(inst)
            bb.instructions[:] = out


def build_body(
    nc, tc, d1, d2, y,
    data_pool, data2_pool, e_pool, f_pool, pm_pool, hse_pool, hpm_pool,
    small_pool, stats_pool, epi_pool,
):
    # persistent per-core stat arrays, one column per row-block.
    # bnst*: (count, mean, M2) per block-column, written two rows per
    # bn_stats instruction (see bn_pair below).
    bnst1 = stats_pool.tile([128, NCOLS, 3], F32, tag="bnst1", name="bnst1")
    bnst2 = stats_pool.tile([128, NCOLS, 3], F32, tag="bnst2", name="bnst2")
    sePair = stats_pool.tile([128, 2, NCOLS], F32, tag="sePair", name="sePair")
    pmPair = stats_pool.tile([128, 2, NCOLS], F32, tag="pmPair", name="pmPair")

    def pool_tt(out, in0, in1, op):
        nc.gpsimd.tensor_tensor(out=out, in0=in0, in1=in1, op=op)

    def bn_pair(t, bnst, col0):
        """One bn_stats over rows (col0, col0+1) of the supertile: the
        input AP interleaves the two 128-col rows element-by-element
        (transpose puts the block dim fastest), so the instruction's
        even elements are row A and its odd elements are row B.  The HW
        computes (count, mean, count*var) for each parity — per-row
        stats, two rows per instruction.  The [128, 2, 3] output AP
        lands them as (count, mean, M2) per block-column.  Emitted raw:
        the bass wrapper's segment-shape assert predates this AP use."""
        in_ap = t[:, col0 % NB : col0 % NB + 2, :].transpose([0, 2, 1])
        out_ap = bnst[:, col0 : col0 + 2, :]
        nc.vector.add_instruction(
            mybir.InstBNStats(
                name=nc.get_next_instruction_name(),
                ins=[nc.vector.lower_ap(in_ap)],
                outs=[nc.vector.lower_ap(out_ap)],
            )
        )

    def fold_tail(src_h1, out_cols, chain):
        """src_h1 [128,2,NB,64] fp16 -> out_cols [128,2,NB] f32 via two
        fp16 2x halving adds + one segmented reduce."""
        hp = hse_pool if chain == "se" else hpm_pool
        h2 = hp.tile([128, 2, NB, 32], FP16, tag=f"h2{chain}", name=f"h2{chain}")
        _tt(nc, h2, src_h1[:, :, :, 0:32], src_h1[:, :, :, 32:64], Alu.add)
        h3 = hp.tile([128, 2, NB, 16], FP16, tag=f"h3{chain}", name=f"h3{chain}")
        _tt(nc, h3, h2[:, :, :, 0:16], h2[:, :, :, 16:32], Alu.add)
        nc.vector.reduce_sum(out=out_cols, in_=h3, axis=mybir.AxisListType.X)

    def stage_load(st):
        rows = slice(st * ST_ROWS, (st + 1) * ST_ROWS)
        src1 = d1[rows, :].rearrange("(p b) n -> p b n", p=128)
        src2 = d2[rows, :].rearrange("(p b) n -> p b n", p=128)
        t1 = data_pool.tile([128, NB, N], F32, tag="t1", name="t1")
        t2 = data2_pool.tile([128, NB, N], F32, tag="t2", name="t2")
        nc.sync.dma_start(out=t1, in_=src1)
        nc.sync.dma_start(out=t2, in_=src2)
        live[("t", st)] = (t1, t2)

    def stage_main(st):
        t1, t2 = live[("t", st)]
        e12 = e_pool.tile([128, 2, NB, N], FP16, tag="e12", name="e12")
        nc.scalar.activation(out=e12[:, 0], in_=t1, func=Act.Exp)
        nc.scalar.activation(out=e12[:, 1], in_=t2, func=Act.Exp)
        pm = pm_pool.tile([128, 2, NB, N], FP16, tag="pm", name="pm")
        pool_tt(pm[:, 0], t1, t2, Alu.mult)
        for k in range(NB // 2):
            bn_pair(t1, bnst1, st * NB + 2 * k)
            bn_pair(t2, bnst2, st * NB + 2 * k)
        live[("e", st)] = (e12, pm)

    def stage_se(st):
        """se1/se2 via one direct fp16 segmented reduce, then c."""
        e12, pm = live[("e", st)]
        cols = slice(st * NB, (st + 1) * NB)
        nc.vector.reduce_sum(
            out=sePair[:, :, cols], in_=e12, axis=mybir.AxisListType.X
        )
        rse2 = small_pool.tile([128, NB], F32, tag="rse2", name="rse2")
        nc.vector.reciprocal(out=rse2, in_=sePair[:, 1, cols])
        cC = small_pool.tile([128, NB], F32, tag="cC", name="cC")
        _tt(nc, cC, sePair[:, 0, cols], rse2, Alu.mult)
        live[("c", st)] = cC

    def stage_f(st):
        """ln(c), then f = c*e2 = Exp(d2 + ln c) per block; f overwrites
        e12 channel 1 (e2's last reader was the se reduce)."""
        cC = live.pop(("c", st))
        _t1, t2 = live[("t", st)]
        e12, pm = live[("e", st)]
        lnc = small_pool.tile([128, NB], F32, tag="lnc", name="lnc")
        nc.scalar.activation(out=lnc, in_=cC, func=Act.Ln)
        f = f_pool.tile([128, NB, N], FP16, tag="f", name="f")
        for b in range(NB):
            nc.scalar.activation(
                out=f[:, b], in_=t2[:, b], func=Act.Exp,
                bias=lnc[:, b : b + 1],
            )
        live[("f", st)] = f

    def stage_pm_chain(st):
        """min then one direct fp16 reduce of the [p12, min] pack."""
        e12, pm = live.pop(("e", st))
        f = live.pop(("f", st))
        live.pop(("t", st))
        cols = slice(st * NB, (st + 1) * NB)
        _tt(nc, pm[:, 1], e12[:, 0], f, Alu.min)
        nc.vector.reduce_sum(
            out=pmPair[:, :, cols], in_=pm, axis=mybir.AxisListType.X
        )

    live = {}
    pre = [None]

    def epi_post():
        _emit_epilogue_post(nc, epi_pool, pre[0], pmPair, y)

    def valid(st):
        return 0 <= st <= NST - 1

    # lags: load k (LEAD cycles ahead of compute) | exp/p12/bn k-LEAD |
    # se k-LEAD-1 | f+min+pm k-LEAD-2
    LEAD = 1
    for k in range(NST + LEAD + 2):
        if valid(k - LEAD - 2) and "f" not in ABLATE:
            stage_f(k - LEAD - 2)
        if k < NST:
            stage_load(k)
        if valid(k - LEAD):
            stage_main(k - LEAD)
        if valid(k - LEAD - 1) and "se" not in ABLATE:
            stage_se(k - LEAD - 1)
        if k == NST + LEAD and not ABLATE:
            pre[0] = _emit_epilogue_pre(nc, epi_pool, bnst1, bnst2, sePair)
        if valid(k - LEAD - 2) and "pm" not in ABLATE and "f" not in ABLATE:
            stage_pm_chain(k - LEAD - 2)

    # ---- per-row epilogue on [128, NCOLS] stat arrays ----
    # (epi_pre was emitted into the pipeline drain above)
    if ABLATE:
        part = epi_pool.tile([128, 1], F32, tag="part", name="part")
        nc.vector.memset(part, 0.0)
        nc.sync.dma_start(out=y[:, :], in_=part)
    else:
        epi_post()


def _emit_epilogue_pre(nc, epi_pool, bnst1, bnst2, sePair):
    """Loss-chain ops that depend only on bn stats + se sums: emitted
    into the pipeline drain so they overlap the last pm chains."""
    def ep(name):
        return epi_pool.tile([128, NCOLS], F32, tag=name, name=name)

    Alu = mybir.AluOpType
    Act = mybir.ActivationFunctionType
    m1, m2_1 = bnst1[:, :, 1], bnst1[:, :, 2]
    m2, m2_2 = bnst2[:, :, 1], bnst2[:, :, 2]
    mm = ep("mm")
    _tt(nc, mm, m1, m2, Alu.mult)
    # den = sqrt(M2_1*M2_2) + (N-1)*eps ; sqrt via exp(0.5*ln(w)) so every
    # ACT func stays in the natural_log_exp_and_others table
    w, sqw = ep("w"), ep("sqw")
    _tt(nc, w, m2_1, m2_2, Alu.mult)
    nc.scalar.activation(out=sqw, in_=w, func=Act.Ln)
    nc.scalar.activation(out=sqw, in_=sqw, func=Act.Exp, scale=0.5)
    den, rden = ep("den"), ep("rden")
    nc.vector.tensor_scalar(
        out=den, in0=sqw, scalar1=(N - 1) * EPS, scalar2=None, op0=Alu.add
    )
    nc.vector.reciprocal(out=rden, in_=den)
    rse1 = ep("rse1")
    nc.vector.reciprocal(out=rse1, in_=sePair[:, 0, :])
    ln_bias = epi_pool.tile([128, 1], F32, tag="ln_bias", name="ln_bias")
    nc.vector.memset(ln_bias, 1.0 + EPS)
    return mm, rden, rse1, ln_bias


def _emit_epilogue_post(nc, epi_pool, pre, pmPair, y):
    def ep(name):
        return epi_pool.tile([128, NCOLS], F32, tag=name, name=name)

    Alu = mybir.AluOpType
    Act = mybir.ActivationFunctionType
    mm, rden, rse1, ln_bias = pre
    s12A = pmPair[:, 0, :]
    MA = pmPair[:, 1, :]

    # num = s12 - N*m1*m2 ; cor = (num + N*eps^2) * rden
    num, cor = ep("num"), ep("cor")
    nc.vector.scalar_tensor_tensor(
        out=num, in0=mm, scalar=-float(N), in1=s12A, op0=Alu.mult, op1=Alu.add
    )
    nc.vector.scalar_tensor_tensor(
        out=cor, in0=num, scalar=float(N) * EPS * EPS, in1=rden,
        op0=Alu.add, op1=Alu.mult,
    )
    c2, cor3 = ep("c2"), ep("cor3")
    _tt(nc, c2, cor, cor, Alu.mult)
    _tt(nc, cor3, c2, cor, Alu.mult)

    # a = |cor3| ; tl1 = ln2 - ln(cor3 + 1 + eps)
    aa, lg, tl1 = ep("aa"), ep("lg"), ep("tl1")
    nc.scalar.activation(out=aa, in_=cor3, func=Act.Abs)
    nc.scalar.activation(out=lg, in_=cor3, func=Act.Ln, bias=ln_bias)
    nc.vector.tensor_scalar(
        out=tl1, in0=lg, scalar1=-1.0, scalar2=float(np.log(2.0)),
        op0=Alu.mult, op1=Alu.add,
    )

    # tl2 = (2/N)*(1 - M/se1)
    vv, tl2 = ep("vv"), ep("tl2")
    _tt(nc, vv, MA, rse1, Alu.mult)
    nc.vector.tensor_scalar(
        out=tl2, in0=vv, scalar1=-2.0 / N, scalar2=2.0 / N,
        op0=Alu.mult, op1=Alu.add,
    )

    # loss = tl2 + a*(tl1 - tl2)
    dd, pp, loss = ep("dd"), ep("pp"), ep("loss")
    _tt(nc, dd, tl1, tl2, Alu.subtract)
    _tt(nc, pp, aa, dd, Alu.mult)
    _tt(nc, loss, tl2, pp, Alu.add)

    part = epi_pool.tile([128, 1], F32, tag="part", name="part")
    nc.vector.reduce_sum(out=part, in_=loss, axis=mybir.AxisListType.X)
    nc.sync.dma_start(out=y[:, :], in_=part)


def _enter_pools(stack, tc):
    names_bufs = [
        ("data", 4), ("data2", 6), ("e", 4), ("f", 2), ("pm", 4),
        ("hse", 1), ("hpm", 1), ("small", 4), ("stats", 1), ("epi", 1),
    ]
    return [
        stack.enter_context(tc.tile_pool(name=nm, bufs=bf))
        for nm, bf in names_bufs
    ]


def _build_program():
    from contextlib import ExitStack

    nc = bass.Bass()
    d1 = nc.dram_tensor("d1", [R, N], F32, kind="ExternalInput")
    d2 = nc.dram_tensor("d2", [R, N], F32, kind="ExternalInput")
    y = nc.dram_tensor("y", [128, 1], F32, kind="ExternalOutput")

    with tile.TileContext(nc) as tc:
        with ExitStack() as stack:
            pools = _enter_pools(stack, tc)
            build_body(nc, tc, d1, d2, y, *pools)

    split_waits(nc)
    return nc


_NC = None
_RUNNER = None


def _get_nc():
    global _NC
    if _NC is None:
        _NC = _build_program()
    return _NC


def _get_runner():
    """Compile the 8-core pjrt executable once and reuse across calls."""
    global _RUNNER
    if _RUNNER is not None:
        return _RUNNER
    import jax
    from jax.sharding import Mesh, PartitionSpec
    from jax.experimental.shard_map import shard_map
    from concourse.bass2jax import (
        _bass_exec_p,
        install_neuronx_cc_hook,
        partition_id_tensor,
    )

    install_neuronx_cc_hook()
    nc = _get_nc()
    partition_name = nc.partition_id_tensor.name if nc.partition_id_tensor else None
    in_names, out_names, out_avals, zero_outs = [], [], [], []
    for alloc in nc.m.functions[0].allocations:
        if not isinstance(alloc, mybir.MemoryLocationSet):
            continue
        name = alloc.memorylocations[0].name
        if alloc.kind == "ExternalInput":
            if name != partition_name:
                in_names.append(name)
        elif alloc.kind == "ExternalOutput":
            out_names.append(name)
            shape = tuple(alloc.tensor_shape)
            dtype = mybir.dt.np(alloc.dtype)
            out_avals.append(jax.core.ShapedArray(shape, dtype))
            zero_outs.append(np.zeros(shape, dtype))
    n_params = len(in_names)
    all_in_names = list(in_names) + out_names
    if partition_name is not None:
        all_in_names.append(partition_name)

    def _body(*args):
        operands = list(args)
        if partition_name is not None:
            operands.append(partition_id_tensor())
        outs = _bass_exec_p.bind(
            *operands,
            out_avals=tuple(out_avals),
            in_names=tuple(all_in_names),
            out_names=tuple(out_names),
            lowering_input_output_aliases=(),
            sim_require_finite=True,
            sim_require_nnan=True,
            nc=nc,
        )
        return tuple(outs)

    devices = jax.devices()[:N_CORES]
    mesh = Mesh(np.asarray(devices), ("core",))
    n_outs = len(out_names)
    in_specs = (PartitionSpec("core"),) * (n_params + n_outs)
    out_specs = (PartitionSpec("core"),) * n_outs
    sharded = jax.jit(
        shard_map(
            _body, mesh=mesh, in_specs=in_specs, out_specs=out_specs,
            check_rep=False,
        ),
        keep_unused=True,
    )
    zero_cat = [
        np.zeros((N_CORES * z.shape[0], *z.shape[1:]), z.dtype) for z in zero_outs
    ]

    def run(d1, d2):
        ins = {"d1": d1, "d2": d2}
        out = sharded(*(ins[nm] for nm in in_names), *zero_cat)
        y = np.asarray(out[out_names.index("y")])
        return y

    _RUNNER = run
    return _RUNNER


def kernel(distribution1, distribution2):
    d1 = np.ascontiguousarray(np.asarray(distribution1, dtype=np.float32))
    d2 = np.ascontiguousarray(np.asarray(distribution2, dtype=np.float32))
    assert d1.shape == (B, N) and d2.shape == (B, N)
    y = _get_runner()(d1, d2)  # [N_CORES*128, 1] partial sums
    return np.asarray([np.sum(y.astype(np.float64))], dtype=np.float32)


# revision 50
# speedup vs baseline: 1.1319x; 1.0245x over previous
"""CORLoss Trainium2 kernel (final).

Reference (per row of N=128):
    mean1 = mean(d1) + EPS ; mean2 = mean(d2) + EPS
    std1, std2 unbiased ; cov = sum((d1-mean1)*(d2-mean2))/(n-1)
    cor  = (cov / (std1*std2 + EPS)) ** 3
    tl1  = -log((cor + 1 + EPS)/2)
    tl2  = mean(|softmax(d1) - softmax(d2)|)
    a = |cor| ; loss_row = a*tl1 + (1-a)*tl2
    out  = sum(loss_row) over all B rows, shape (1,)

Data-parallel over 8 NeuronCores, 16384 rows/core, streamed as
[128 partitions, 16 blocks, 128] supertiles (one row per (partition,
block)), software-pipelined with lags load k | exp/p12/bn k-1 |
se k-2 | f/min/s12/M k-3.

Per supertile:
  DMA : both streams on the SP HWDGE ring (an ACT-ring trigger blocks
        the ACT queue).
  ACT : e1 = Exp(d1), e2 = Exp(d2) in fp16; f = c*e2 computed per
        128-col block as Exp(d2 + ln c) - rows == partitions inside one
        block, so ln c is a legal per-partition bias.  All ACT funcs
        (Exp/Ln/Abs) live in the natural_log_exp_and_others table.
  DVE : per-row (mean, M2) of both raw streams via two-row bn_stats -
        the input AP interleaves two rows element-by-element (transpose
        puts the block-pair dim fastest), so the instruction's even/odd
        statistics ARE the two rows' stats; emitted raw because the
        bass wrapper's shape assert predates this AP use.  se1/se2 and
        s12/M are single segmented fp16 reduces of the packed [e1,e2]
        and [p12, min] tiles (HW runs 16-bit TensorReduce ~4x; the cost
        model prices it 1x - ignore the simulator here).  min(e1, f)
        uses Σ|a-b| = Σa + Σb - 2Σmin(a,b) (a,b >= 0):
            T = Σ|e1 - c*e2| = 2*se1 - 2*Σmin(e1, c*e2)
        so there is no Abs pass and no subtraction stream.
  Pool: p12 = d1*d2 (fp16 out) - its only legal ops are tt add/sub/mult.

The per-row epilogue turns (mean, M2, se1, s12, M) into cor, tl1, tl2
and one [128,1] partial sum per core (sqrt via exp(0.5*ln w) to stay in
one ACT table); the host sums the 8*128 partials in float64.
"""

import sys

sys.path.insert(0, "/opt/trn_rl_repo")

import numpy as np

import concourse.bass as bass
import concourse.tile as tile
from concourse import mybir

B, N = 131072, 128
EPS = 1e-3
N_CORES = 8
R = B // N_CORES          # rows per core = 16384
ST_ROWS = 2048            # rows per supertile
NB = ST_ROWS // 128       # 16 row-blocks per supertile
NST = R // ST_ROWS        # 8 supertiles per core
NCOLS = R // 128          # 128 stat columns per core
F32 = mybir.dt.float32
FP16 = mybir.dt.float16
Alu = mybir.AluOpType
Act = mybir.ActivationFunctionType


ABLATE = set()  # timing probes only; populated externally


def _tt(nc, out, a, b, op):
    nc.vector.tensor_tensor(out=out, in0=a, in1=b, op=op)


def split_waits(nc, cap=1):
    """This walrus build rejects instructions carrying more than ~1 inline
    semaphore wait; move excess waits onto fresh same-engine nops placed
    immediately before the instruction."""
    for fn in nc.m.functions:
        for bb in fn.blocks:
            snapshot = list(bb.instructions)
            out = []
            for inst in snapshot:
                si = inst.sync_info
                if si is not None and si.on_wait and len(si.on_wait) > cap:
                    waits = list(si.on_wait)
                    extra, keep = waits[:-cap], waits[-cap:]
                    while si.on_wait:
                        si.on_wait.pop()
                    for w in keep:
                        si.on_wait.append(w)
                    for w in extra:
                        bi = nc.engines[inst.engine].nop(nofuse=True, hint="wsplit")
                        nop_inst = bi.ins
                        for fb in nc.m.functions[0].blocks:
                            if fb.instructions and fb.instructions[-1] is nop_inst:
                                fb.instructions.pop()
                                break
                        nop_inst.sync_info = mybir.SyncInfo(on_wait=[w], on_update=[])
                        out.append(nop_inst)
                out.append(inst)
            bb.instructions[:] = out


def build_body(
    nc, tc, d1, d2, y,
    data_pool, data2_pool, e_pool, f_pool, pm_pool, hse_pool, hpm_pool,
    small_pool, stats_pool, epi_pool,
):
    # persistent per-core stat arrays, one column per row-block.
    # bnst*: (count, mean, M2) per block-column, written two rows per
    # bn_stats instruction (see bn_pair below).
    bnst1 = stats_pool.tile([128, NCOLS, 3], F32, tag="bnst1", name="bnst1")
    bnst2 = stats_pool.tile([128, NCOLS, 3], F32, tag="bnst2", name="bnst2")
    sePair = stats_pool.tile([128, 2, NCOLS], F32, tag="sePair", name="sePair")
    pmPair = stats_pool.tile([128, 2, NCOLS], F32, tag="pmPair", name="pmPair")

    def pool_tt(out, in0, in1, op):
        nc.gpsimd.tensor_tensor(out=out, in0=in0, in1=in1, op=op)

    def bn_pair(t, bnst, col0):
        """One bn_stats over rows (col0, col0+1) of the supertile: the
        input AP interleaves the two 128-col rows element-by-element
        (transpose puts the block dim fastest), so the instruction's
        even elements are row A and its odd elements are row B.  The HW
        computes (count, mean, count*var) for each parity — per-row
        stats, two rows per instruction.  The [128, 2, 3] output AP
        lands them as (count, mean, M2) per block-column.  Emitted raw:
        the bass wrapper's segment-shape assert predates this AP use."""
        in_ap = t[:, col0 % NB : col0 % NB + 2, :].transpose([0, 2, 1])
        out_ap = bnst[:, col0 : col0 + 2, :]
        nc.vector.add_instruction(
            mybir.InstBNStats(
                name=nc.get_next_instruction_name(),
                ins=[nc.vector.lower_ap(in_ap)],
                outs=[nc.vector.lower_ap(out_ap)],
            )
        )

    def fold_tail(src_h1, out_cols, chain):
        """src_h1 [128,2,NB,64] fp16 -> out_cols [128,2,NB] f32 via two
        fp16 2x halving adds + one segmented reduce."""
        hp = hse_pool if chain == "se" else hpm_pool
        h2 = hp.tile([128, 2, NB, 32], FP16, tag=f"h2{chain}", name=f"h2{chain}")
        _tt(nc, h2, src_h1[:, :, :, 0:32], src_h1[:, :, :, 32:64], Alu.add)
        h3 = hp.tile([128, 2, NB, 16], FP16, tag=f"h3{chain}", name=f"h3{chain}")
        _tt(nc, h3, h2[:, :, :, 0:16], h2[:, :, :, 16:32], Alu.add)
        nc.vector.reduce_sum(out=out_cols, in_=h3, axis=mybir.AxisListType.X)

    def stage_load(st):
        rows = slice(st * ST_ROWS, (st + 1) * ST_ROWS)
        src1 = d1[rows, :].rearrange("(p b) n -> p b n", p=128)
        src2 = d2[rows, :].rearrange("(p b) n -> p b n", p=128)
        t1 = data_pool.tile([128, NB, N], F32, tag="t1", name="t1")
        t2 = data2_pool.tile([128, NB, N], F32, tag="t2", name="t2")
        nc.sync.dma_start(out=t1, in_=src1)
        nc.sync.dma_start(out=t2, in_=src2)
        live[("t", st)] = (t1, t2)

    def stage_main(st):
        t1, t2 = live[("t", st)]
        e12 = e_pool.tile([128, 2, NB, N], FP16, tag="e12", name="e12")
        nc.scalar.activation(out=e12[:, 0], in_=t1, func=Act.Exp)
        nc.scalar.activation(out=e12[:, 1], in_=t2, func=Act.Exp)
        pm = pm_pool.tile([128, 2, NB, N], FP16, tag="pm", name="pm")
        pool_tt(pm[:, 0], t1, t2, Alu.mult)
        for k in range(NB // 2):
            bn_pair(t1, bnst1, st * NB + 2 * k)
            bn_pair(t2, bnst2, st * NB + 2 * k)
        live[("e", st)] = (e12, pm)

    def stage_se(st):
        """se1/se2 via one direct fp16 segmented reduce, then c."""
        e12, pm = live[("e", st)]
        cols = slice(st * NB, (st + 1) * NB)
        nc.vector.reduce_sum(
            out=sePair[:, :, cols], in_=e12, axis=mybir.AxisListType.X
        )
        rse2 = small_pool.tile([128, NB], F32, tag="rse2", name="rse2")
        nc.vector.reciprocal(out=rse2, in_=sePair[:, 1, cols])
        cC = small_pool.tile([128, NB], F32, tag="cC", name="cC")
        _tt(nc, cC, sePair[:, 0, cols], rse2, Alu.mult)
        live[("c", st)] = cC

    def stage_f(st):
        """ln(c), then f = c*e2 = Exp(d2 + ln c) per block; f overwrites
        e12 channel 1 (e2's last reader was the se reduce)."""
        cC = live.pop(("c", st))
        _t1, t2 = live[("t", st)]
        e12, pm = live[("e", st)]
        lnc = small_pool.tile([128, NB], F32, tag="lnc", name="lnc")
        nc.scalar.activation(out=lnc, in_=cC, func=Act.Ln)
        f = f_pool.tile([128, NB, N], FP16, tag="f", name="f")
        for b in range(NB):
            nc.scalar.activation(
                out=f[:, b], in_=t2[:, b], func=Act.Exp,
                bias=lnc[:, b : b + 1],
            )
        live[("f", st)] = f

    def stage_pm_chain(st):
        """min then one direct fp16 reduce of the [p12, min] pack."""
        e12, pm = live.pop(("e", st))
        f = live.pop(("f", st))
        live.pop(("t", st))
        cols = slice(st * NB, (st + 1) * NB)
        _tt(nc, pm[:, 1], e12[:, 0], f, Alu.min)
        nc.vector.reduce_sum(
            out=pmPair[:, :, cols], in_=pm, axis=mybir.AxisListType.X
        )

    live = {}
    pre = [None]

    def epi_post():
        _emit_epilogue_post(nc, epi_pool, pre[0], pmPair, y)

    def valid(st):
        return 0 <= st <= NST - 1

    # lags: load k (LEAD cycles ahead of compute) | exp/p12/bn k-LEAD |
    # se k-LEAD-1 | f+min+pm k-LEAD-2
    LEAD = 1
    for k in range(NST + LEAD + 2):
        if valid(k - LEAD - 2) and "f" not in ABLATE:
            stage_f(k - LEAD - 2)
        if k < NST:
            stage_load(k)
        if valid(k - LEAD):
            stage_main(k - LEAD)
        if valid(k - LEAD - 1) and "se" not in ABLATE:
            stage_se(k - LEAD - 1)
        if k == NST + LEAD and not ABLATE:
            pre[0] = _emit_epilogue_pre(nc, epi_pool, bnst1, bnst2, sePair)
        if valid(k - LEAD - 2) and "pm" not in ABLATE and "f" not in ABLATE:
            stage_pm_chain(k - LEAD - 2)

    # ---- per-row epilogue on [128, NCOLS] stat arrays ----
    # (epi_pre was emitted into the pipeline drain above)
    if ABLATE:
        part = epi_pool.tile([128, 1], F32, tag="part", name="part")
        nc.vector.memset(part, 0.0)
        nc.sync.dma_start(out=y[:, :], in_=part)
    else:
        epi_post()


def _emit_epilogue_pre(nc, epi_pool, bnst1, bnst2, sePair):
    """Loss-chain ops that depend only on bn stats + se sums: emitted
    into the pipeline drain so they overlap the last pm chains."""
    def ep(name):
        return epi_pool.tile([128, NCOLS], F32, tag=name, name=name)

    Alu = mybir.AluOpType
    Act = mybir.ActivationFunctionType
    m1, m2_1 = bnst1[:, :, 1], bnst1[:, :, 2]
    m2, m2_2 = bnst2[:, :, 1], bnst2[:, :, 2]
    mm = ep("mm")
    _tt(nc, mm, m1, m2, Alu.mult)
    # den = sqrt(M2_1*M2_2) + (N-1)*eps ; sqrt via exp(0.5*ln(w)) so every
    # ACT func stays in the natural_log_exp_and_others table
    w, sqw = ep("w"), ep("sqw")
    _tt(nc, w, m2_1, m2_2, Alu.mult)
    nc.scalar.activation(out=sqw, in_=w, func=Act.Ln)
    nc.scalar.activation(out=sqw, in_=sqw, func=Act.Exp, scale=0.5)
    den, rden = ep("den"), ep("rden")
    nc.vector.tensor_scalar(
        out=den, in0=sqw, scalar1=(N - 1) * EPS, scalar2=None, op0=Alu.add
    )
    nc.vector.reciprocal(out=rden, in_=den)
    rse1 = ep("rse1")
    nc.vector.reciprocal(out=rse1, in_=sePair[:, 0, :])
    ln_bias = epi_pool.tile([128, 1], F32, tag="ln_bias", name="ln_bias")
    nc.vector.memset(ln_bias, 1.0 + EPS)
    return mm, rden, rse1, ln_bias


def _emit_epilogue_post(nc, epi_pool, pre, pmPair, y):
    def ep(name):
        return epi_pool.tile([128, NCOLS], F32, tag=name, name=name)

    Alu = mybir.AluOpType
    Act = mybir.ActivationFunctionType
    mm, rden, rse1, ln_bias = pre
    s12A = pmPair[:, 0, :]
    MA = pmPair[:, 1, :]

    # num = s12 - N*m1*m2 ; cor = (num + N*eps^2) * rden
    num, cor = ep("num"), ep("cor")
    nc.vector.scalar_tensor_tensor(
        out=num, in0=mm, scalar=-float(N), in1=s12A, op0=Alu.mult, op1=Alu.add
    )
    nc.vector.scalar_tensor_tensor(
        out=cor, in0=num, scalar=float(N) * EPS * EPS, in1=rden,
        op0=Alu.add, op1=Alu.mult,
    )
    c2, cor3 = ep("c2"), ep("cor3")
    _tt(nc, c2, cor, cor, Alu.mult)
    _tt(nc, cor3, c2, cor, Alu.mult)

    # a = |cor3| ; tl1 = ln2 - ln(cor3 + 1 + eps)
    aa, lg, tl1 = ep("aa"), ep("lg"), ep("tl1")
    nc.scalar.activation(out=aa, in_=cor3, func=Act.Abs)
    nc.scalar.activation(out=lg, in_=cor3, func=Act.Ln, bias=ln_bias)
    nc.vector.tensor_scalar(
        out=tl1, in0=lg, scalar1=-1.0, scalar2=float(np.log(2.0)),
        op0=Alu.mult, op1=Alu.add,
    )

    # tl2 = (2/N)*(1 - M/se1)
    vv, tl2 = ep("vv"), ep("tl2")
    _tt(nc, vv, MA, rse1, Alu.mult)
    nc.vector.tensor_scalar(
        out=tl2, in0=vv, scalar1=-2.0 / N, scalar2=2.0 / N,
        op0=Alu.mult, op1=Alu.add,
    )

    # loss = tl2 + a*(tl1 - tl2)
    dd, pp, loss = ep("dd"), ep("pp"), ep("loss")
    _tt(nc, dd, tl1, tl2, Alu.subtract)
    _tt(nc, pp, aa, dd, Alu.mult)
    _tt(nc, loss, tl2, pp, Alu.add)

    part = epi_pool.tile([128, 1], F32, tag="part", name="part")
    nc.vector.reduce_sum(out=part, in_=loss, axis=mybir.AxisListType.X)
    nc.sync.dma_start(out=y[:, :], in_=part)


def _enter_pools(stack, tc):
    names_bufs = [
        ("data", 4), ("data2", 6), ("e", 4), ("f", 2), ("pm", 4),
        ("hse", 1), ("hpm", 1), ("small", 4), ("stats", 1), ("epi", 1),
    ]
    return [
        stack.enter_context(tc.tile_pool(name=nm, bufs=bf))
        for nm, bf in names_bufs
    ]


def _build_program():
    from contextlib import ExitStack

    nc = bass.Bass()
    d1 = nc.dram_tensor("d1", [R, N], F32, kind="ExternalInput")
    d2 = nc.dram_tensor("d2", [R, N], F32, kind="ExternalInput")
    y = nc.dram_tensor("y", [128, 1], F32, kind="ExternalOutput")

    with tile.TileContext(nc) as tc:
        with ExitStack() as stack:
            pools = _enter_pools(stack, tc)
            build_body(nc, tc, d1, d2, y, *pools)

    split_waits(nc)
    return nc


_NC = None
_RUNNER = None


def _get_nc():
    global _NC
    if _NC is None:
        _NC = _build_program()
    return _NC


def _get_runner():
    """Compile the 8-core pjrt executable once and reuse across calls."""
    global _RUNNER
    if _RUNNER is not None:
        return _RUNNER
    import jax
    from jax.sharding import Mesh, PartitionSpec
    from jax.experimental.shard_map import shard_map
    from concourse.bass2jax import (
        _bass_exec_p,
        install_neuronx_cc_hook,
        partition_id_tensor,
    )

    install_neuronx_cc_hook()
    nc = _get_nc()
    partition_name = nc.partition_id_tensor.name if nc.partition_id_tensor else None
    in_names, out_names, out_avals, zero_outs = [], [], [], []
    for alloc in nc.m.functions[0].allocations:
        if not isinstance(alloc, mybir.MemoryLocationSet):
            continue
        name = alloc.memorylocations[0].name
        if alloc.kind == "ExternalInput":
            if name != partition_name:
                in_names.append(name)
        elif alloc.kind == "ExternalOutput":
            out_names.append(name)
            shape = tuple(alloc.tensor_shape)
            dtype = mybir.dt.np(alloc.dtype)
            out_avals.append(jax.core.ShapedArray(shape, dtype))
            zero_outs.append(np.zeros(shape, dtype))
    n_params = len(in_names)
    all_in_names = list(in_names) + out_names
    if partition_name is not None:
        all_in_names.append(partition_name)

    def _body(*args):
        operands = list(args)
        if partition_name is not None:
            operands.append(partition_id_tensor())
        outs = _bass_exec_p.bind(
            *operands,
            out_avals=tuple(out_avals),
            in_names=tuple(all_in_names),
            out_names=tuple(out_names),
            lowering_input_output_aliases=(),
            sim_require_finite=True,
            sim_require_nnan=True,
            nc=nc,
        )
        return tuple(outs)

    devices = jax.devices()[:N_CORES]
    mesh = Mesh(np.asarray(devices), ("core",))
    n_outs = len(out_names)
    in_specs = (PartitionSpec("core"),) * (n_params + n_outs)
    out_specs = (PartitionSpec("core"),) * n_outs
    sharded = jax.jit(
        shard_map(
            _body, mesh=mesh, in_specs=in_specs, out_specs=out_specs,
            check_rep=False,
        ),
        keep_unused=True,
    )
    zero_cat = [
        np.zeros((N_CORES * z.shape[0], *z.shape[1:]), z.dtype) for z in zero_outs
    ]

    def run(d1, d2):
        ins = {"d1": d1, "d2": d2}
        out = sharded(*(ins[nm] for nm in in_names), *zero_cat)
        y = np.asarray(out[out_names.index("y")])
        return y

    _RUNNER = run
    return _RUNNER


def kernel(distribution1, distribution2):
    d1 = np.ascontiguousarray(np.asarray(distribution1, dtype=np.float32))
    d2 = np.ascontiguousarray(np.asarray(distribution2, dtype=np.float32))
    assert d1.shape == (B, N) and d2.shape == (B, N)
    y = _get_runner()(d1, d2)  # [N_CORES*128, 1] partial sums
    return np.asarray([np.sum(y.astype(np.float64))], dtype=np.float32)
